# Initial kernel scaffold
#
"""Trainium2 Bass kernel for nn_D_FullAttention (B=8, L=S=2048, H=2, E=64).

Strategy: data-parallel over batch B across the 8 NeuronCores (one batch per
core).  Per core, a flash-attention-style pipeline per (head, l-chunk):

  - Q, K, V are loaded as natural (seq, h*e) tiles and PE-transposed once into
    (h*e, seq) layout so the e-contraction sits on the partition dim.
  - scores are computed TRANSPOSED: S^T[s, l] = sum_e K[s,e] Q[l,e]
    (lhsT = K^T tile, rhs = Q^T chunk), so the softmax axis (s) is the
    partition dim.
  - softmax skips the max subtraction (fp32-safe here; softmax is invariant to
    the reference's per-batch deg-max shift).  The degradation bias never
    touches the exp stream: exp(0.125*S + d_s) = exp(0.125*S) * exp(d_s), and
    the exp(d_s) factor is pre-multiplied into the V-side weights.
  - The denominator comes for free from a ones-column appended to V:
    U^T = [V;1]^T_aug(scaled) @ expS^T accumulates (65, l) in PSUM where row
    64 is the softmax denominator.
  - U^T is PE-transposed back to (l, 65); a per-partition reciprocal multiply
    normalizes, then the (l, 64) tiles are gathered and DMA'd per chunk to the
    natural output layout.
  - The degradation MLP runs on the transposed V (which doubles as the MLP
    input since vflat^T == V^T tiles), chunk by chunk, producing deg in
    (s mod 128, s//128) layout; sigmoid is computed via the exp table so the
    ACT engine never swaps activation tables.
  - All loads and the MLP are software-pipelined (emission-interleaved) into
    the first attention chunk's s-loop so the ACT exp stream starts ~7us in
    and never starves.

All big matmuls use float32r (full-rate on TRN2 for moving dim >= 256).
"""

import numpy as np
from contextlib import ExitStack

import concourse.bass as bass
import concourse.tile as tile
from concourse import bacc, mybir
from concourse.bass_utils import run_bass_kernel_spmd
from concourse.masks import make_identity

F32 = mybir.dt.float32
F32R = mybir.dt.float32r
AF = mybir.ActivationFunctionType
ALU = mybir.AluOpType

B = 8
L = 2048          # query length (== S, the key length)
H = 2
E = 64
F = H * E         # 128
NT = L // 128     # 16 seq tiles of 128
LCHUNK = 1024     # l processed per (head, chunk) iteration
NLC = L // LCHUNK
NHALF = LCHUNK // 512
SCALE = 1.0 / 8.0  # 1/sqrt(E)

_CACHE = {}


def _emit_kernel(nc, tc, ctx, q, k, v, W1, b1, W2, b2, o, dbg_out=None):
    res = ctx.enter_context(tc.tile_pool(name="res", bufs=1))
    nat = ctx.enter_context(tc.tile_pool(name="nat", bufs=12))
    expp = ctx.enter_context(tc.tile_pool(name="expp", bufs=6))
    voutp = ctx.enter_context(tc.tile_pool(name="voutp", bufs=2))
    outp = ctx.enter_context(tc.tile_pool(name="outp", bufs=4))
    psS = ctx.enter_context(tc.tile_pool(name="psS", bufs=2, space="PSUM"))
    psV = ctx.enter_context(tc.tile_pool(name="psV", bufs=1, space="PSUM"))
    psO = ctx.enter_context(tc.tile_pool(name="psO", bufs=2, space="PSUM"))

    ident = res.tile([128, 128], F32, tag="ident")
    make_identity(nc, ident)

    # ---- resident SBUF tensors ----
    qT = res.tile([128, L], F32R, tag="qT")    # rows h*64+e, cols l
    kT = res.tile([128, L], F32R, tag="kT")    # rows h*64+e, cols s
    vT = res.tile([128, L], F32R, tag="vT")    # rows h*64+e, cols s (MLP input)
    hidT = res.tile([65, L], F32, tag="hidT")  # MLP hidden^T + ones row 64
    # vaug layout: [h=2][st=16][c=65]; c 0:64 = v[s,h,:], c 64 = 1.0
    vaug = res.tile([128, H * NT * 65], F32, tag="vaug")
    # vaug scaled by exp(0.125*deg[s]) row-wise (folds the deg bias into AV)
    vaug_sc = res.tile([128, H * NT * 65], F32R, tag="vaug_sc")
    W1s = res.tile([128, 64], F32R, tag="W1s")
    b1s = res.tile([64, 1], F32, tag="b1s")
    W2a = res.tile([65, 1], F32, tag="W2a")   # [W2; b2]
    deg = res.tile([128, NT], F32, tag="deg")  # deg[p, t] for s = t*128+p

    # ---- constants ----
    nc.vector.memset(hidT[64:65, :], 1.0)
    vaug_4d = vaug.rearrange("p (h st c) -> p h st c", h=H, c=65)
    vaug_sc_4d = vaug_sc.rearrange("p (h st c) -> p h st c", h=H, c=65)
    nc.gpsimd.memset(vaug_4d[:, :, :, 64:65], 1.0)
    expdeg = res.tile([128, NT], F32, tag="expdeg")

    nc.gpsimd.dma_start(out=W1s[:, :], in_=W1.bitcast(F32R))
    nc.gpsimd.dma_start(out=b1s[:, :], in_=b1.rearrange("(e o) -> e o", o=1))
    nc.gpsimd.dma_start(out=W2a[0:64, :], in_=W2)
    nc.gpsimd.dma_start(out=W2a[64:65, :], in_=b2.rearrange("(e o) -> e o", o=1))

    qf = q.rearrange("l h e -> l (h e)")
    kf = k.rearrange("l h e -> l (h e)")
    vf = v.rearrange("l h e -> l (h e)")

    # one DMA loads 4 natural 128x128 tiles; then 4 PE transposes + DVE copies
    def load_dma4(src_flat, g):
        n4 = nat.tile([128, 512], F32, tag="nat", name="n4")
        src = src_flat[g * 512:(g + 1) * 512, :].rearrange(
            "(a p) f -> p a f", p=128
        )
        nc.sync.dma_start(out=n4.rearrange("p (a f) -> p a f", a=4), in_=src)
        return n4

    def transpose_tiles(n4, dstT, g, aa):
        for a in aa:
            t = g * 4 + a
            ps = psO.tile([128, 128], F32, tag="ps", name="ps")
            nc.tensor.transpose(ps[:, :], n4[:, a * 128:(a + 1) * 128], ident[:, :])
            nc.vector.tensor_copy(dstT[:, t * 128:(t + 1) * 128], ps[:, :])

    def load_transposed4(src_flat, dstT, g):
        transpose_tiles(load_dma4(src_flat, g), dstT, g, range(4))

    # ---- V path + MLP (produces expdeg / vaug_sc for the AV matmuls) ----
    def v_chunk(g):
        load_transposed4(vf, vT, g)
        for t in range(4 * g, 4 * g + 4):
            nc.gpsimd.dma_start(
                out=vaug_4d[:, :, t, 0:64],
                in_=v[t * 128:(t + 1) * 128, :, :],
            )
        hp = psO.tile([64, 512], F32, tag="ps")
        nc.tensor.matmul(
            hp[:, :], lhsT=W1s[:, :], rhs=vT[:, g * 512:(g + 1) * 512],
            start=True, stop=True,
        )
        nc.vector.tensor_scalar(
            hidT[0:64, g * 512:(g + 1) * 512], hp[:, :], b1s[:, :], 0.0,
            ALU.add, ALU.max,
        )
        lg = psO.tile([128, 4], F32, tag="ps")
        for a in range(4):
            t = 4 * g + a
            nc.tensor.matmul(
                lg[:, a:a + 1], lhsT=hidT[:, t * 128:(t + 1) * 128],
                rhs=W2a[:, :], start=True, stop=True,
            )
        # sigmoid(x) = 1/(1+exp(-x)) via the exp table (avoids ACT table swaps)
        dg = deg[:, 4 * g:4 * g + 4]
        nc.scalar.activation(dg, lg[:, :], AF.Exp, scale=-1.0)
        nc.vector.tensor_scalar_add(dg, dg, 1.0)
        nc.vector.reciprocal(dg, dg)
        nc.vector.tensor_scalar(dg, dg, 0.01, 0.99, ALU.max, ALU.min)
        nc.vector.tensor_scalar_mul(dg, dg, SCALE)
        nc.scalar.activation(expdeg[:, 4 * g:4 * g + 4], dg, AF.Exp)
        for t in range(4 * g, 4 * g + 4):
            nc.vector.tensor_scalar_mul(
                vaug_sc_4d[:, :, t, :], vaug_4d[:, :, t, :],
                expdeg[:, t:t + 1],
            )


    # ---- first-needed loads; the rest interleave into chunk 0's s-loop ----
    load_transposed4(kf, kT, 0)
    load_transposed4(qf, qT, 0)
    load_transposed4(qf, qT, 1)
    load_hooks = {
        0: lambda: v_chunk(0),
        2: lambda: load_transposed4(kf, kT, 1),
        3: lambda: v_chunk(1),
        6: lambda: load_transposed4(kf, kT, 2),
        7: lambda: v_chunk(2),
        10: lambda: load_transposed4(kf, kT, 3),
        11: lambda: v_chunk(3),
        13: lambda: load_transposed4(qf, qT, 2),
        14: lambda: load_transposed4(qf, qT, 3),
    }

    # ---- attention ----
    # vout row 64 already holds 1/denom; after transpose, po col 64 is the
    # per-l reciprocal, used directly as the per-partition scalar.
    def emit_output_j(st8, j, use_act=False):
        h, l0, w, vout, otb = st8
        po = psO.tile([128, 65], F32, tag="ps")
        nc.tensor.transpose(
            po[:, :], vout[:, j * 128:(j + 1) * 128], ident[0:65, 0:65]
        )
        nc.vector.tensor_scalar_mul(
            otb[:, j * 64:(j + 1) * 64], po[:, 0:64], po[:, 64:65]
        )

    def emit_output_dma(st8):
        h, l0, w, vout, otb = st8
        dst = o[l0:l0 + w, h, :].rearrange("(a p) e -> p a e", p=128)
        nc.gpsimd.dma_start(
            out=dst, in_=otb[:, 0:w // 2].rearrange("p (a e) -> p a e", e=64)
        )

    def emit_output(st8):
        # last chunk: pipeline in halves (transposes/muls/DMA overlap)
        h, l0, w, vout, otb = st8
        nj = w // 256
        for hf in (0, 1):
            for j in range(hf * nj, (hf + 1) * nj):
                emit_output_j(st8, j)
            dst = o[l0 + hf * w // 2:l0 + (hf + 1) * w // 2, h, :].rearrange(
                "(a p) e -> p a e", p=128
            )
            nc.sync.dma_start(
                out=dst,
                in_=otb[:, hf * w // 4:(hf + 1) * w // 4].rearrange(
                    "p (a e) -> p a e", e=64
                ),
            )

    chunks = [(0, 0, 1024), (0, 1024, 1024), (1, 0, 1024), (1, 1024, 1024)]
    prev = None
    for ci, (h, l0, w) in enumerate(chunks):
        vtp = psV.tile([65, LCHUNK], F32, tag="vtp")
        for st in range(NT):
            if ci == 0 and st in load_hooks:
                load_hooks[st]()
            if prev is not None and 1 <= st <= prev[2] // 128:
                emit_output_j(prev, st - 1)
                if st == prev[2] // 128:
                    emit_output_dma(prev)
                    prev = None
            sp = psS.tile([128, LCHUNK], F32, tag="sp")
            es = expp.tile([128, LCHUNK], F32R, tag="es")

            def qk(nh):
                nc.tensor.matmul(
                    sp[:, nh * 512:(nh + 1) * 512],
                    lhsT=kT[h * 64:h * 64 + 64, st * 128:(st + 1) * 128],
                    rhs=qT[h * 64:h * 64 + 64,
                           l0 + nh * 512:l0 + (nh + 1) * 512],
                    start=True, stop=True,
                )

            for nh in range(w // 512):
                qk(nh)
            nc.scalar.activation(es[:, 0:w], sp[:, 0:w], AF.Exp, scale=SCALE)
            for nh in range(w // 512):
                nc.tensor.matmul(
                    vtp[:, nh * 512:(nh + 1) * 512],
                    lhsT=vaug_sc[:, (h * NT + st) * 65:(h * NT + st) * 65 + 65],
                    rhs=es[:, nh * 512:(nh + 1) * 512],
                    start=(st == 0), stop=(st == NT - 1),
                )
        vout = voutp.tile([65, LCHUNK], F32, tag="vout")
        last = (h, l0) == (chunks[-1][0], chunks[-1][1])
        # copy all 65 rows so vtp's PSUM banks free up after one read, then
        # reciprocal in SBUF in place
        if last:
            for hf in (0, 1):
                sl = slice(hf * w // 2, (hf + 1) * w // 2)
                nc.scalar.copy(vout[:, sl], vtp[:, sl])
                nc.vector.reciprocal(vout[64:65, sl], vout[64:65, sl])
        else:
            nc.vector.tensor_copy(vout[:, 0:w], vtp[:, 0:w])
            nc.vector.reciprocal(vout[64:65, 0:w], vout[64:65, 0:w])
        otb = outp.tile([128, LCHUNK // 2], F32, tag="otb")
        assert prev is None
        prev = (h, l0, w, vout, otb)
    emit_output(prev)
    if dbg_out is not None:
        nc.sync.dma_start(out=dbg_out[0], in_=deg[:, :])
        nc.sync.dma_start(out=dbg_out[1], in_=hidT[:, :])
        nc.sync.dma_start(out=dbg_out[2], in_=vT[:, :].bitcast(F32))
        nc.sync.dma_start(out=dbg_out[3], in_=qT[:, :].bitcast(F32))
        nc.sync.dma_start(out=dbg_out[4], in_=kT[:, :].bitcast(F32))


def build(dbg=False):
    if "nc" in _CACHE:
        return _CACHE["nc"]
    nc = bacc.Bacc("TRN2", target_bir_lowering=False, debug=False, num_devices=B)
    q = nc.dram_tensor("q", (L, H, E), F32, kind="ExternalInput").ap()
    k = nc.dram_tensor("k", (L, H, E), F32, kind="ExternalInput").ap()
    v = nc.dram_tensor("v", (L, H, E), F32, kind="ExternalInput").ap()
    W1 = nc.dram_tensor("W1", (F, 64), F32, kind="ExternalInput").ap()
    b1 = nc.dram_tensor("b1", (64,), F32, kind="ExternalInput").ap()
    W2 = nc.dram_tensor("W2", (64, 1), F32, kind="ExternalInput").ap()
    b2 = nc.dram_tensor("b2", (1,), F32, kind="ExternalInput").ap()
    o = nc.dram_tensor("o", (L, H, E), F32, kind="ExternalOutput").ap()
    dbg_out = None
    if dbg:
        dbg_out = (nc.dram_tensor("dbg", (128, NT), F32, kind="ExternalOutput").ap(),
                   nc.dram_tensor("dbg_hid", (65, L), F32, kind="ExternalOutput").ap(),
                   nc.dram_tensor("dbg_vT", (128, L), F32, kind="ExternalOutput").ap(),
                   nc.dram_tensor("dbg_qT", (128, L), F32, kind="ExternalOutput").ap(),
                   nc.dram_tensor("dbg_kT", (128, L), F32, kind="ExternalOutput").ap())
    with tile.TileContext(nc) as tc, ExitStack() as ctx:
        _emit_kernel(nc, tc, ctx, q, k, v, W1, b1, W2, b2, o, dbg_out)
    nc.compile()
    _CACHE["nc"] = nc
    return nc


def run(inputs, trace=False):
    nc = build()
    c = np.ascontiguousarray
    in_maps = [
        {
            "q": c(inputs["queries"][b]).astype(np.float32),
            "k": c(inputs["keys"][b]).astype(np.float32),
            "v": c(inputs["values"][b]).astype(np.float32),
            "W1": c(inputs["W1"]).astype(np.float32),
            "b1": c(inputs["b1"]).astype(np.float32),
            "W2": c(inputs["W2"]).astype(np.float32),
            "b2": c(inputs["b2"]).astype(np.float32),
        }
        for b in range(B)
    ]
    try:
        res = run_bass_kernel_spmd(nc, in_maps, core_ids=list(range(B)), trace=trace)
    except ModuleNotFoundError:
        res = run_bass_kernel_spmd(nc, in_maps, core_ids=list(range(B)), trace=False)
    out = np.stack([res.results[b]["o"] for b in range(B)])
    return out, res


def kernel(**inputs) -> np.ndarray:
    out, _ = run(inputs, trace=False)
    return out



# revision 4
# speedup vs baseline: 1.0098x; 1.0098x over previous
"""Trainium2 Bass kernel for nn_D_FullAttention (B=8, L=S=2048, H=2, E=64).

Data-parallel over batch B across 8 NeuronCores (one batch per core).

Per-core pipeline (redesigned for TimelineSim engine balance):

  - Q, K loaded natural, PE-transposed once, kept resident in SBUF as bf16
    (e on partitions).  V kept three ways: natural bf16 + ones column
    ("vaug", the AV rhs), transposed f32r (MLP input), natural f32 staging.
  - scores computed TRANSPOSED per (head, l-chunk, s-tile): S^T[s, l] in
    PSUM; softmax axis s on partitions.
  - The degradation bias rides inside the exp activation itself:
    es = exp(SCALE*S^T + dg[s]) via the ACT engine's per-partition bias AP
    (dg = SCALE*clip(sigmoid(MLP(v)))).  Max-subtraction skipped (softmax
    shift-invariant, fp32-safe range).
  - A fraction of exp tiles runs on the DVE instead, via a Schraudolph
    bit-trick: bits_i16 = round(S^T*A + (dg*128/ln2 + 16256 - C)), bitcast
    to bf16 (~2-3% sawtooth rel err on those tiles only) - this balances
    ACT vs DVE since ACT exp is otherwise the bottleneck.
  - AV streams NATURAL V: out[l-tile, 65] accumulates over s-tiles with
    lhsT = es 128-col slice (full 128-row PE util), rhs = vaug[s-tile]
    (64 v cols + ones col -> denominator lands in column 64).  Output is
    produced in natural (l, h*e) layout - no output transposes.
  - Epilogue per chunk: DVE reciprocal of col 64 + per-l-tile scale, then
    natural-layout DMA stores (both heads per row range in one DMA).
"""

import numpy as np
from contextlib import ExitStack

import concourse.bass as bass
import concourse.tile as tile
from concourse import bacc, mybir
from concourse.bass_utils import run_bass_kernel_spmd
from concourse.masks import make_identity

F32 = mybir.dt.float32
F32R = mybir.dt.float32r
BF16 = mybir.dt.bfloat16
I16 = mybir.dt.int16
AF = mybir.ActivationFunctionType
ALU = mybir.AluOpType

B = 8
L = 2048
H = 2
E = 64
F = H * E          # 128
NT = L // 128      # 16 s-tiles
LCHUNK = 1024
SCALE = 1.0 / 8.0  # 1/sqrt(E)

LN2 = float(np.log(2.0))
SCH_A = SCALE * 128.0 / LN2          # score -> bf16-bit slope
SCH_C = 5.53                          # centers the sawtooth (RN convert)
SCH_D = 127.0 * 128.0 - SCH_C
SCH_DEG = 128.0 / LN2                 # dg (already *SCALE) -> bits

# (chunk, st) pairs whose exp runs on DVE via the bit-trick.  Chunk 0 is
# kept light on DVE (it is busy with transpose copies there).
import os
_NODVE = os.environ.get("NODVE", "0") == "1"
DVE_STS = {
    0: {6, 12},
    1: {2, 4, 6, 9, 11, 13},
    2: {2, 4, 6, 9, 11, 13},
    3: {2, 4, 6, 9, 11, 13},
}
if _NODVE:
    DVE_STS = {0: set(), 1: set(), 2: set(), 3: set()}

_CACHE = {}


def _emit_kernel(nc, tc, ctx, q, k, v, W1, b1, W2, b2, o):
    res = ctx.enter_context(tc.tile_pool(name="res", bufs=1))
    natp = ctx.enter_context(tc.tile_pool(name="nat", bufs=6))
    esp = ctx.enter_context(tc.tile_pool(name="esp", bufs=5))
    otbp = ctx.enter_context(tc.tile_pool(name="otbp", bufs=2))
    r4p = ctx.enter_context(tc.tile_pool(name="r4p", bufs=2))
    psS = ctx.enter_context(tc.tile_pool(name="psS", bufs=2, space="PSUM"))
    psT = ctx.enter_context(tc.tile_pool(name="psT", bufs=2, space="PSUM"))
    psA = ctx.enter_context(tc.tile_pool(name="psA", bufs=2, space="PSUM"))

    ident = res.tile([128, 128], F32, tag="ident")
    make_identity(nc, ident)

    # ---- resident SBUF tensors ----
    qT = res.tile([128, L], BF16, tag="qT")    # rows h*64+e, cols l
    kT = res.tile([128, L], BF16, tag="kT")    # rows h*64+e, cols s
    vT = res.tile([128, L], F32R, tag="vT")    # MLP input (f32r)
    hidT = res.tile([65, L], F32, tag="hidT")  # MLP hidden^T + ones row 64
    # vaug: natural V in bf16 + ones col: [p, h, st, c], c 64 == 1.0
    vaug = res.tile([128, H * NT * 65], BF16, tag="vaug")
    W1s = res.tile([128, 64], F32R, tag="W1s")
    b1s = res.tile([64, 1], F32, tag="b1s")
    W2a = res.tile([65, 1], F32, tag="W2a")    # [W2; b2]
    dg = res.tile([128, NT], F32, tag="dg")    # SCALE*clip(sigmoid), s=t*128+p
    dgD = res.tile([128, NT], F32, tag="dgD")  # Schraudolph bias bits

    nc.vector.memset(hidT[64:65, :], 1.0)
    vaug_4d = vaug.rearrange("p (h st c) -> p h st c", h=H, c=65)
    nc.gpsimd.memset(vaug_4d[:, :, :, 64:65], 1.0)

    nc.gpsimd.dma_start(out=W1s[:, :], in_=W1.bitcast(F32R))
    nc.gpsimd.dma_start(out=b1s[:, :], in_=b1.rearrange("(e o) -> e o", o=1))
    nc.gpsimd.dma_start(out=W2a[0:64, :], in_=W2)
    nc.gpsimd.dma_start(out=W2a[64:65, :], in_=b2.rearrange("(e o) -> e o", o=1))

    qf = q.rearrange("l h e -> l (h e)")
    kf = k.rearrange("l h e -> l (h e)")
    vf = v.rearrange("l h e -> l (h e)")

    def load_nat(src_flat, g):
        n4 = natp.tile([128, 512], F32, tag="nat", name="n4")
        src = src_flat[g * 512:(g + 1) * 512, :].rearrange(
            "(a p) f -> p a f", p=128
        )
        nc.sync.dma_start(out=n4.rearrange("p (a f) -> p a f", a=4), in_=src)
        return n4

    def transpose4(n4, dstT, g):
        for a in range(4):
            t = g * 4 + a
            pt = psT.tile([128, 512], F32, tag="pt", name="pt")
            nc.tensor.transpose(
                pt[:, 0:128], n4[:, a * 128:(a + 1) * 128], ident[:, :]
            )
            nc.vector.tensor_copy(dstT[:, t * 128:(t + 1) * 128], pt[:, 0:128])

    # ---- V path: vT (f32r, MLP), vaug (bf16 natural), dg/dgD ----
    def v_mlp(n4v, g):
        transpose4(n4v, vT, g)
        n4v_4d = n4v.rearrange("p (a h e) -> p a h e", a=4, h=H)
        for a in range(4):
            t = 4 * g + a
            nc.gpsimd.tensor_copy(
                vaug_4d[:, :, t, 0:64], n4v_4d[:, a, :, :]
            )
        hp = psT.tile([64, 512], F32, tag="pt", name="hp")
        nc.tensor.matmul(
            hp[:, :], lhsT=W1s[:, :], rhs=vT[:, g * 512:(g + 1) * 512],
            start=True, stop=True,
        )
        nc.vector.tensor_scalar(
            hidT[0:64, g * 512:(g + 1) * 512], hp[:, :], b1s[:, :], 0.0,
            ALU.add, ALU.max,
        )
        lg = psT.tile([128, 4], F32, tag="pt", name="lg")
        for a in range(4):
            t = 4 * g + a
            nc.tensor.matmul(
                lg[:, a:a + 1], lhsT=hidT[:, t * 128:(t + 1) * 128],
                rhs=W2a[:, :], start=True, stop=True,
            )
        dgs = dg[:, 4 * g:4 * g + 4]
        # sigmoid(x) = 1/(1+exp(-x)) via exp table (no ACT table swap)
        nc.scalar.activation(dgs, lg[:, :], AF.Exp, scale=-1.0)
        nc.vector.tensor_scalar_add(dgs, dgs, 1.0)
        nc.vector.reciprocal(dgs, dgs)
        nc.vector.tensor_scalar(dgs, dgs, 0.01, 0.99, ALU.max, ALU.min)
        nc.vector.tensor_scalar_mul(dgs, dgs, SCALE)
        nc.vector.tensor_scalar(
            dgD[:, 4 * g:4 * g + 4], dgs, SCH_DEG, SCH_D, ALU.mult, ALU.add
        )

    # ---- prologue: loads ordered for fastest first exp ----
    n4v0 = load_nat(vf, 0)
    n4k0 = load_nat(kf, 0)
    n4q0 = load_nat(qf, 0)
    n4q1 = load_nat(qf, 1)
    n4v1 = load_nat(vf, 1)
    n4k1 = load_nat(kf, 1)
    v_mlp(n4v0, 0)
    transpose4(n4k0, kT, 0)
    transpose4(n4q0, qT, 0)
    transpose4(n4q1, qT, 1)

    pend = {"v1": n4v1, "k1": n4k1}

    def hook_v1():
        v_mlp(pend["v1"], 1)

    def hook_k1():
        transpose4(pend["k1"], kT, 1)

    load_hooks = {
        0: lambda: pend.update(v2=load_nat(vf, 2)),
        1: hook_v1,
        2: hook_k1,
        3: lambda: pend.update(k2=load_nat(kf, 2)),
        4: lambda: v_mlp(pend["v2"], 2),
        5: lambda: pend.update(v3=load_nat(vf, 3)),
        6: lambda: transpose4(pend["k2"], kT, 2),
        7: lambda: pend.update(k3=load_nat(kf, 3)),
        8: lambda: v_mlp(pend["v3"], 3),
        10: lambda: transpose4(pend["k3"], kT, 3),
        11: lambda: pend.update(q2=load_nat(qf, 2)),
        12: lambda: transpose4(pend["q2"], qT, 2),
        13: lambda: pend.update(q3=load_nat(qf, 3)),
        14: lambda: transpose4(pend["q3"], qT, 3),
    }

    # ---- attention main loop ----
    chunks = [(0, 0), (1, 0), (0, 1024), (1, 1024)]
    otb = None
    for ci, (h, l0) in enumerate(chunks):
        if h == 0:
            otb = otbp.tile([128, 8, H, 64], F32, tag="otb", name="otb")
        accA = psA.tile([128, 260], F32, tag="acc", name="accA")
        accB = psA.tile([128, 260], F32, tag="acc", name="accB")
        dve_sts = DVE_STS[ci]
        for st in range(NT):
            if ci == 0 and st in load_hooks:
                load_hooks[st]()
            sp = psS.tile([128, LCHUNK], F32, tag="sp")
            for nh in range(2):
                nc.tensor.matmul(
                    sp[:, nh * 512:(nh + 1) * 512],
                    lhsT=kT[h * 64:h * 64 + 64, st * 128:(st + 1) * 128],
                    rhs=qT[h * 64:h * 64 + 64,
                           l0 + nh * 512:l0 + (nh + 1) * 512],
                    start=True, stop=True,
                )
            es = esp.tile([128, LCHUNK], BF16, tag="es")
            if st in dve_sts:
                nc.vector.tensor_scalar(
                    es[:, :].bitcast(I16), sp[:, :], SCH_A,
                    dgD[:, st:st + 1], ALU.mult, ALU.add,
                )
            else:
                nc.scalar.activation(
                    es[:, :], sp[:, :], AF.Exp,
                    bias=dg[:, st:st + 1], scale=SCALE,
                )
            for j in range(8):
                acc = accA if j < 4 else accB
                off = (j % 4) * 65
                nc.tensor.matmul(
                    acc[:, off:off + 65],
                    lhsT=es[:, j * 128:(j + 1) * 128],
                    rhs=vaug_4d[:, h, st, :],
                    start=(st == 0), stop=(st == NT - 1),
                )
        # ---- chunk epilogue: normalize into natural otb ----
        r4 = r4p.tile([128, 8], F32, tag="r4")
        accA_r = accA.rearrange("p (j c) -> p j c", c=65)
        accB_r = accB.rearrange("p (j c) -> p j c", c=65)
        nc.vector.reciprocal(r4[:, 0:4], accA_r[:, :, 64])
        nc.vector.reciprocal(r4[:, 4:8], accB_r[:, :, 64])
        for j in range(8):
            acc = accA if j < 4 else accB
            off = (j % 4) * 65
            nc.vector.tensor_scalar_mul(
                otb[:, j, h, :], acc[:, off:off + 64], r4[:, j:j + 1]
            )
        if h == 1:
            dst = o[l0:l0 + LCHUNK, :, :].rearrange(
                "(a p) h e -> p a h e", p=128
            )
            nc.sync.dma_start(out=dst, in_=otb[:, :, :, :])


def build():
    if "nc" in _CACHE:
        return _CACHE["nc"]
    nc = bacc.Bacc("TRN2", target_bir_lowering=False, debug=False, num_devices=B)
    q = nc.dram_tensor("q", (L, H, E), F32, kind="ExternalInput").ap()
    k = nc.dram_tensor("k", (L, H, E), F32, kind="ExternalInput").ap()
    v = nc.dram_tensor("v", (L, H, E), F32, kind="ExternalInput").ap()
    W1 = nc.dram_tensor("W1", (F, 64), F32, kind="ExternalInput").ap()
    b1 = nc.dram_tensor("b1", (64,), F32, kind="ExternalInput").ap()
    W2 = nc.dram_tensor("W2", (64, 1), F32, kind="ExternalInput").ap()
    b2 = nc.dram_tensor("b2", (1,), F32, kind="ExternalInput").ap()
    o = nc.dram_tensor("o", (L, H, E), F32, kind="ExternalOutput").ap()
    with tile.TileContext(nc) as tc, ExitStack() as ctx:
        _emit_kernel(nc, tc, ctx, q, k, v, W1, b1, W2, b2, o)
    nc.compile()
    _CACHE["nc"] = nc
    return nc


def run(inputs, trace=False):
    nc = build()
    c = np.ascontiguousarray
    in_maps = [
        {
            "q": c(inputs["queries"][b]).astype(np.float32),
            "k": c(inputs["keys"][b]).astype(np.float32),
            "v": c(inputs["values"][b]).astype(np.float32),
            "W1": c(inputs["W1"]).astype(np.float32),
            "b1": c(inputs["b1"]).astype(np.float32),
            "W2": c(inputs["W2"]).astype(np.float32),
            "b2": c(inputs["b2"]).astype(np.float32),
        }
        for b in range(B)
    ]
    try:
        res = run_bass_kernel_spmd(nc, in_maps, core_ids=list(range(B)), trace=trace)
    except ModuleNotFoundError:
        res = run_bass_kernel_spmd(nc, in_maps, core_ids=list(range(B)), trace=False)
    out = np.stack([res.results[b]["o"] for b in range(B)])
    return out, res


def kernel(**inputs) -> np.ndarray:
    out, _ = run(inputs, trace=False)
    return out
